# revision 13
# baseline (speedup 1.0000x reference)
"""RoIAlign (scale_and_translate, linear, antialias) Trainium2 kernel.

Strategy: channel-shard across 8 NeuronCores. Each core receives a
contiguous [512, 512, 8] slice of the feature map (kept resident in
SBUF), plus host-precomputed dense resampling weight matrices
Wy/Wx [512, 32] per box, and computes all 512 boxes for its 8 channels:

  stage 1 (PE):  T[i, x, c]   = sum_y Wy[y, i] * F[y, x, c]   (per 128-px x-tile)
  transpose:     SBUF->SBUF DMA rearrange  T -> rhs2[x, (i, c)]
  stage 2 (PE):  out[j, i, c] = sum_x Wx[x, j] * T[i, x, c]

Device output layout is [n, j, i, c]; the host transposes to
[n, i, j, c] and concatenates channel shards.
"""

import numpy as np

H = 512
W = 512
C = 64
N_BOXES = 512
OUT = 32
N_CORES = 8
C_LOC = C // N_CORES  # 8 channels per core
PART = 128


# ---------------------------------------------------------------------------
# Host-side weight computation (mirrors jax.image.scale_and_translate with
# method="linear", antialias=True)
# ---------------------------------------------------------------------------

def _compute_weight_mat(in_size, out_size, scale, translation):
    inv_scale = 1.0 / scale
    kernel_scale = max(inv_scale, 1.0)
    sample_f = (np.arange(out_size, dtype=np.float64) + 0.5) * inv_scale \
        - translation * inv_scale - 0.5
    x = np.abs(sample_f[None, :] - np.arange(in_size, dtype=np.float64)[:, None]) \
        / kernel_scale
    weights = np.maximum(0.0, 1.0 - x)
    total = weights.sum(axis=0, keepdims=True)
    weights = np.where(
        np.abs(total) > 1000.0 * float(np.finfo(np.float32).eps),
        weights / np.where(total != 0, total, 1.0),
        0.0,
    )
    valid = (sample_f >= -0.5) & (sample_f <= in_size - 0.5)
    return np.where(valid[None, :], weights, 0.0).astype(np.float32)


def host_geometry(boxes):
    """Per-box dense weights + extents.

    Returns wy_all [N, 512, OUT], wx_all [N, 512, OUT] fp32 and a list of
    per-box geometry dicts.
    """
    boxes = np.asarray(boxes, dtype=np.float64)
    wy_all = np.zeros((N_BOXES, H, OUT), np.float32)
    wx_all = np.zeros((N_BOXES, W, OUT), np.float32)
    geoms = []
    for n in range(N_BOXES):
        cx, cy, w, h = boxes[n]
        x0 = cx - w / 2
        y0 = cy - h / 2
        w = max(w, 1e-6)
        h = max(h, 1e-6)
        x_scale = OUT / (w * W)
        y_scale = OUT / (h * H)
        ty = -y0 * OUT / h
        tx = -x0 * OUT / w
        wy = _compute_weight_mat(H, OUT, y_scale, ty)
        wx = _compute_weight_mat(W, OUT, x_scale, tx)
        wy_all[n] = wy
        wx_all[n] = wx

        ynz = np.nonzero(wy.any(axis=1))[0]
        xnz = np.nonzero(wx.any(axis=1))[0]
        if len(ynz) == 0 or len(xnz) == 0:
            geoms.append(None)
            continue
        r0, r1 = int(ynz[0]), int(ynz[-1]) + 1
        c0, c1 = int(xnz[0]), int(xnz[-1]) + 1
        # y-slices within 128-row partition tiles. matmul operands must
        # start at partition 0/32/64, and K depth is free on PE, so each
        # slice spans partitions [0, hi) of its tile (zero weights above
        # r0 contribute nothing).
        yslices = []
        for t in range(H // PART):
            lo = max(r0, t * PART)
            hi = min(r1, (t + 1) * PART)
            if lo < hi:
                yslices.append((t, hi - t * PART))
        xts = list(range(c0 // PART, (c1 - 1) // PART + 1))
        geoms.append({"yslices": yslices, "xts": xts})
    return wy_all, wx_all, geoms


def reference_numpy(feature_map, wy_all, wx_all):
    """Two-stage numpy computation used to validate the host weights."""
    out = np.zeros((N_BOXES, OUT, OUT, feature_map.shape[2]), np.float32)
    f = feature_map
    for n in range(N_BOXES):
        t = np.einsum("yi,yxc->ixc", wy_all[n], f)
        out[n] = np.einsum("xj,ixc->ijc", wx_all[n], t)
    return out


# ---------------------------------------------------------------------------
# Device program
# ---------------------------------------------------------------------------

def _split_multiwait_bir(raw: bytes) -> bytes:
    """The walrus build here accepts only one sync wait per instruction.
    Hoist extra waits onto single-wait EventSemaphore instructions inserted
    just before, on the same engine (per-engine order is preserved)."""
    import orjson

    d = orjson.loads(raw)
    ctr = 0
    for fn in d.get("functions", []):
        for bb in fn.get("blocks") or []:
            out = []
            for ins in bb["instructions"]:
                si = ins.get("sync_info")
                ws = (si or {}).get("on_wait") or []
                if len(ws) > 1:
                    for w in ws[:-1]:
                        ctr += 1
                        out.append({
                            "debug": ins.get("debug", 0),
                            "engine": ins["engine"],
                            "ins": [],
                            "outs": [],
                            "name": f"{ins['name']}-xw{ctr}",
                            "opcode": "EventSemaphore",
                            "sync_info": {"on_update": [], "on_wait": [w]},
                        })
                    si["on_wait"] = [ws[-1]]
                out.append(ins)
            bb["instructions"] = out
    return orjson.dumps(d)


def _patch_serialization(nc):
    orig = nc.to_json_bytes

    def patched():
        return _split_multiwait_bir(orig())

    nc.to_json_bytes = patched
    return nc

def _build_program(geoms):
    import concourse.bass as bass
    import concourse.mybir as mybir
    import concourse.tile as tile
    from concourse.vector_clock import ScopedClock
    import bass_rust

    class TC(tile.TileContext):
        """TileContext with the tail drain's multi-sem wait split into
        individual single-wait instructions (this walrus rejects >1 wait
        on a CTRL instruction)."""

        def _drain_and_barrier(self, tick_clock, wait_clock):
            nc = self.nc
            probe = nc.sync.drain()
            wait_clock.add_sem_waits(
                probe.ins, ScopedClock({None: tick_clock.global_clock})
            )
            waits = list(probe.ins.sync_info.on_wait)
            probe.ins.sync_info = bass_rust.SyncInfo(on_wait=[], on_update=[])
            by_name = {hh.name: hh for hh in self.sems.allocated().values()}
            for wt in waits:
                nc.sync.wait_ge(by_name[wt.ant_name], wt.wait_value)
            nc.all_engine_barrier()
            popped = nc._tile_sem_poison_stack.pop()
            assert popped is self._sem_poison
            nc.clear_and_free_semaphores(list(self.sems.allocated().values()))
            nc.all_engine_barrier()

    FP32 = mybir.dt.float32
    BF16 = mybir.dt.bfloat16
    nc = bass.Bass()
    # f is channel-major [c, y, x] bf16 so per-(c, y-tile) lhsT slices have
    # contiguous x columns (FWL-friendly weight loads).
    f_d = nc.dram_tensor("f", [C_LOC, H, W], BF16, kind="ExternalInput")
    wy_d = nc.dram_tensor("wy", [N_BOXES, H, OUT], BF16, kind="ExternalInput")
    wx_d = nc.dram_tensor("wx", [N_BOXES, W, OUT], FP32, kind="ExternalInput")
    # device output layout [n, j, c, i]
    out_d = nc.dram_tensor("out", [N_BOXES, OUT, C_LOC, OUT], FP32,
                           kind="ExternalOutput")

    NT = H // PART          # 4 y/x partition tiles
    CI = C_LOC * OUT        # 256: (c, i) free size of T^T / psum tiles
    CHUNK = 16              # boxes per weight-DMA chunk

    from contextlib import ExitStack

    with TC(nc) as tc, ExitStack() as ctx:
        fpool = ctx.enter_context(tc.tile_pool(name="fmap", bufs=1))
        wpool = ctx.enter_context(tc.tile_pool(name="wts", bufs=2))
        rpool = ctx.enter_context(tc.tile_pool(name="rhs2", bufs=4))
        opool = ctx.enter_context(tc.tile_pool(name="osb", bufs=4))
        p1pool = ctx.enter_context(tc.tile_pool(name="psumT", bufs=4, space="PSUM"))
        p2pool = ctx.enter_context(tc.tile_pool(name="psum2", bufs=2, space="PSUM"))

        # resident feature map slice: [128, (c, t, x)], y = t*128 + p
        f_sb = fpool.tile([PART, C_LOC * NT * W], BF16)
        f_v = f_sb[:].rearrange("p (c t x) -> p c t x", c=C_LOC, t=NT)
        nc.sync.dma_start(
            out=f_v,
            in_=f_d.rearrange("c (t p) x -> p c t x", p=PART),
        )

        evac_flip = 0
        for chunk in range(N_BOXES // CHUNK):
            b0 = chunk * CHUNK
            wy_sb = wpool.tile([PART, CHUNK * NT * OUT], BF16, tag="wy")
            wy_v = wy_sb[:].rearrange("p (b t i) -> p b t i", b=CHUNK, t=NT)
            nc.sync.dma_start(
                out=wy_v,
                in_=wy_d[b0:b0 + CHUNK].rearrange("b (t p) i -> p b t i", p=PART),
            )
            wx_sb = wpool.tile([PART, CHUNK * NT * OUT], FP32, tag="wx")
            wx_v = wx_sb[:].rearrange("p (b t i) -> p b t i", b=CHUNK, t=NT)
            nc.sync.dma_start(
                out=wx_v,
                in_=wx_d[b0:b0 + CHUNK].rearrange("b (t p) i -> p b t i", p=PART),
            )

            for bl in range(CHUNK):
                n = b0 + bl
                g = geoms[n]
                if g is None:
                    continue
                yslices = g["yslices"]
                xts = g["xts"]

                psum2 = p2pool.tile([OUT, CI], FP32)
                for k, xt in enumerate(xts):
                    # stage 1: T^T[x, (c, i)] = sum_y F[y, x, c] * Wy[y, i]
                    psum_t = p1pool.tile([PART, CI], FP32)
                    xsl = slice(xt * PART, (xt + 1) * PART)
                    for c in range(C_LOC):
                        for si, (t, hi) in enumerate(yslices):
                            nc.tensor.matmul(
                                out=psum_t[:, c * OUT:(c + 1) * OUT],
                                lhsT=f_v[0:hi, c, t, xsl],
                                rhs=wy_v[0:hi, bl, t, :],
                                start=(si == 0),
                                stop=(si == len(yslices) - 1),
                            )
                    # evacuate PSUM -> SBUF (alternate DVE / ACT)
                    rhs2 = rpool.tile([PART, CI], FP32)
                    if evac_flip & 1:
                        nc.scalar.copy(rhs2[:], psum_t[:])
                    else:
                        nc.vector.tensor_copy(out=rhs2[:], in_=psum_t[:])
                    evac_flip += 1
                    # stage 2: out[j, (c, i)] += sum_x Wx[x, j] * T^T[x, (c, i)]
                    nc.tensor.matmul(
                        out=psum2[:],
                        lhsT=wx_v[:, bl, xt, :],
                        rhs=rhs2[:],
                        start=(k == 0),
                        stop=(k == len(xts) - 1),
                    )
                o_sb = opool.tile([OUT, CI], FP32)
                if evac_flip & 1:
                    nc.scalar.copy(o_sb[:], psum2[:])
                else:
                    nc.vector.tensor_copy(out=o_sb[:], in_=psum2[:])
                evac_flip += 1
                nc.sync.dma_start(
                    out=out_d[n].rearrange("j c i -> j (c i)"),
                    in_=o_sb[:],
                )
    return _patch_serialization(nc)


# ---------------------------------------------------------------------------
# Entry point
# ---------------------------------------------------------------------------

_LAST = {}


def kernel(feature_map, boxes, output_width):
    from concourse.bass_utils import run_bass_kernel_spmd

    feature_map = np.asarray(feature_map, dtype=np.float32)
    boxes_np = np.asarray(boxes, dtype=np.float32)
    assert int(output_width) == OUT

    wy_all, wx_all, geoms = host_geometry(boxes_np)
    nc = _build_program(geoms)

    import ml_dtypes
    wy_bf = wy_all.astype(ml_dtypes.bfloat16)
    in_maps = []
    for k in range(N_CORES):
        # channel-major [c, y, x] bf16 slice
        f_k = np.ascontiguousarray(
            feature_map[:, :, k * C_LOC:(k + 1) * C_LOC].transpose(2, 0, 1)
        ).astype(ml_dtypes.bfloat16)
        in_maps.append({"f": f_k, "wy": wy_bf, "wx": wx_all})

    _LAST["nc"] = nc
    _LAST["in_maps"] = in_maps
    res = run_bass_kernel_spmd(nc, in_maps, list(range(N_CORES)))

    out = np.empty((N_BOXES, OUT, OUT, C), np.float32)
    for k in range(N_CORES):
        # device layout [n, j, c, i] -> [n, i, j, c]
        out[:, :, :, k * C_LOC:(k + 1) * C_LOC] = \
            res.results[k]["out"].transpose(0, 3, 1, 2)
    return out


def estimate_hw_ns():
    """Cost-model estimate of the per-core kernel duration (ns)."""
    from concourse.timeline_sim import TimelineSim
    nc = _LAST.get("nc")
    if nc is None:
        return -1
    sim = TimelineSim(nc)
    sim.simulate()
    return int(sim.time)


def measure_wall(n=5):
    """Wall-clock of repeated dispatches (includes axon round trips)."""
    import time
    from concourse.bass_utils import run_bass_kernel_spmd
    times = []
    for _ in range(n):
        t0 = time.perf_counter()
        run_bass_kernel_spmd(_LAST["nc"], _LAST["in_maps"], list(range(N_CORES)))
        times.append(time.perf_counter() - t0)
    return times
